# revision 11
# baseline (speedup 1.0000x reference)
"""Trainium2 Bass kernel for nn_BaselineParser (segment-pool + transformer block +
biaffine parser loss), data-parallel over batch across 8 NeuronCores.

Self-contained: hardcodes shapes B=32, S=1024, D=768, F=2048, W=384, H=8.
Each core processes 4 batch rows and returns partial (sum nll*mask, sum mask);
the host combines partials into the scalar loss.

Numerics: matmul path runs in bf16 (weights folded/padded on host), the
"exact path" (masking, -1e9 fill, gold gather, log-softmax, final reductions)
runs in fp32.  The loss is dominated by gold-on-masked-column tokens whose
nll is ~1e9 computed exactly, so bf16 on the matmul path perturbs the loss
only at ~1e-6 relative.
"""

import math
import os
import numpy as np
import ml_dtypes

import concourse.bass as bass
import concourse.tile as tile
from concourse import bacc, mybir
from concourse.bass_utils import run_bass_kernel_spmd

F32 = mybir.dt.float32
BF16 = mybir.dt.bfloat16
I32 = mybir.dt.int32
AF = mybir.ActivationFunctionType
ALU = mybir.AluOpType
AX = mybir.AxisListType

B, S, D, FF = 32, 1024, 768, 2048
W = 384
H = 8
DH = 96
DHP = 128            # padded head dim
NCORES = 8
NB = B // NCORES     # batches per core
NEG = -1.0e9
KD = D // 128        # 6 contraction chunks over D
TC = W // 128        # 3 token chunks
SC = S // 128        # 8 subword chunks


# ---------------------------------------------------------------- host prep

def _prep_host(inp):
    """Fold LN scales + head padding into weight matrices (fp32 math, bf16 out)."""
    f4 = np.float32
    Wqkv = np.asarray(inp['Wqkv'], f4)
    bqkv = np.asarray(inp['bqkv'], f4)
    g1 = np.asarray(inp['ln1_g'], f4)
    b1ln = np.asarray(inp['ln1_b'], f4)
    g2 = np.asarray(inp['ln2_g'], f4)
    b2ln = np.asarray(inp['ln2_b'], f4)

    Wf = g1[:, None] * Wqkv                      # fold ln1 gain
    bf = b1ln @ Wqkv + bqkv                      # fold ln1 bias
    sc = f4(1.0 / math.sqrt(DH))
    Wf[:, :D] *= sc                              # fold 1/sqrt(dh) into Q
    bf[:D] *= sc

    # pad heads 96 -> 128: Q' heads 0..7, K' heads 8..15 -> [768, 2048]
    Wqk = np.zeros((D, 2 * H * DHP), f4)
    bqk = np.zeros((2 * H * DHP,), f4)
    for h in range(H):
        Wqk[:, DHP * h: DHP * h + DH] = Wf[:, DH * h: DH * h + DH]
        bqk[DHP * h: DHP * h + DH] = bf[DH * h: DH * h + DH]
        Wqk[:, DHP * (H + h): DHP * (H + h) + DH] = Wf[:, D + DH * h: D + DH * h + DH]
        bqk[DHP * (H + h): DHP * (H + h) + DH] = bf[D + DH * h: D + DH * h + DH]

    # V' [768, 1024]: head h cols 128h..128h+95, col 128h+96 is the all-ones
    # (colsum) column: zero weights, bias 1.
    Wv = np.zeros((D, H * DHP), f4)
    bv = np.zeros((H * DHP,), f4)
    for h in range(H):
        Wv[:, DHP * h: DHP * h + DH] = Wf[:, 2 * D + DH * h: 2 * D + DH * h + DH]
        bv[DHP * h: DHP * h + DH] = bf[2 * D + DH * h: 2 * D + DH * h + DH]
        bv[DHP * h + DH] = 1.0

    # Wo' [1024, 768]: rows 128h+j <- Wo rows 96h+j, pad rows zero.
    Wo = np.asarray(inp['Wo'], f4)
    Wop = np.zeros((H * DHP, D), f4)
    for h in range(H):
        Wop[DHP * h: DHP * h + DH] = Wo[DH * h: DH * h + DH]

    W1 = np.asarray(inp['W1'], f4)
    b1 = np.asarray(inp['b1'], f4)
    W1f = g2[:, None] * W1
    b1f = b2ln @ W1 + b1

    bf16 = ml_dtypes.bfloat16
    return {
        'wqk': Wqk.astype(bf16), 'bqk': bqk,
        'wv': Wv.astype(bf16), 'bv': bv.astype(bf16),
        'wo': Wop.astype(bf16), 'bo': np.asarray(inp['bo'], f4),
        'w1': W1f.astype(bf16), 'b1': b1f,
        'w2': np.asarray(inp['W2'], f4).astype(bf16),
        'b2': np.asarray(inp['b2'], f4),
        'wbi': np.asarray(inp['Wbi'], f4).astype(bf16),
        'uw': np.asarray(inp['Uw'], f4).astype(bf16),
        'ub': np.asarray(inp['Ub'], f4).reshape(1, 1),
        'root': np.asarray(inp['root'], f4).astype(bf16),
    }


# ---------------------------------------------------------------- bass build

def _declare(nc):
    """Declare per-core DRAM tensors; returns dict of APs."""
    t = {}

    def inp(name, shape, dt):
        t[name] = nc.dram_tensor(name, list(shape), dt, kind="ExternalInput").ap()

    inp('lh', (NB, S, D), BF16)
    inp('wid', (NB, S), I32)
    inp('gold', (NB, W), I32)
    inp('wqk', (D, 2 * H * DHP), BF16)
    inp('bqk', (2 * H * DHP,), F32)
    inp('wv', (D, H * DHP), BF16)
    inp('bv', (H * DHP,), BF16)
    inp('wo', (H * DHP, D), BF16)
    inp('bo', (D,), F32)
    inp('w1', (D, FF), BF16)
    inp('b1', (FF,), F32)
    inp('w2', (FF, D), BF16)
    inp('b2', (D,), F32)
    inp('wbi', (D, D), BF16)
    inp('uw', (D,), BF16)
    inp('ub', (1, 1), F32)
    inp('root', (D,), BF16)
    t['out'] = nc.dram_tensor('out', [1, 2], F32, kind="ExternalOutput").ap()
    return t


def _build_body(nc, tc_, t):
    """Emit the whole per-core program inside TileContext tc_."""
    import contextlib
    ctx = contextlib.ExitStack()
    with ctx:
        _build_body_inner(nc, tc_, t, ctx)


def _build_body_inner(nc, tc_, t, ctx):
    pool = ctx.enter_context
    con = pool(tc_.tile_pool(name="con", bufs=1))
    wbig = pool(tc_.tile_pool(name="wbig", bufs=6))
    wvp = pool(tc_.tile_pool(name="wvp", bufs=6))
    wst = pool(tc_.tile_pool(name="wst", bufs=17))
    lhp = pool(tc_.tile_pool(name="lhp", bufs=8))
    ohp = pool(tc_.tile_pool(name="ohp", bufs=8))
    xfam = pool(tc_.tile_pool(name="xfam", bufs=27))
    zp = pool(tc_.tile_pool(name="zp", bufs=8))
    sqp = pool(tc_.tile_pool(name="sqp", bufs=2))
    qkp = pool(tc_.tile_pool(name="qkp", bufs=5))
    vtp = pool(tc_.tile_pool(name="vtp", bufs=3))
    exp_p = pool(tc_.tile_pool(name="exp_p", bufs=4))
    yp = pool(tc_.tile_pool(name="yp", bufs=8))
    gp = pool(tc_.tile_pool(name="gp", bufs=4))
    t1p = pool(tc_.tile_pool(name="t1p", bufs=7))
    rows = pool(tc_.tile_pool(name="rows", bufs=4))
    batch_rows = pool(tc_.tile_pool(name="batch_rows", bufs=4))
    loss_p = pool(tc_.tile_pool(name="loss_p", bufs=2))
    bcp = pool(tc_.tile_pool(name="bcp", bufs=4))
    tmp_p = pool(tc_.tile_pool(name="tmp_p", bufs=3))

    ps_mm = pool(tc_.tile_pool(name="ps_mm", bufs=2, space="PSUM"))
    ps_acc = pool(tc_.tile_pool(name="ps_acc", bufs=6, space="PSUM"))

    f32r = mybir.dt.float32r
    STAGE = int(os.environ.get('KSTAGE', '99'))

    def _probe(src_ap):
        po = con.tile([1, 2], F32, name="probe", tag="probe")
        n = src_ap.free_size()
        nc.gpsimd.memset(po[:], 0.0)
        nc.vector.tensor_copy(po[:, 0:n], src_ap)
        nc.sync.dma_start(t['out'][:, :], po[:])

    # ---------------- constants
    ones_row = con.tile([1, 128], BF16)
    nc.gpsimd.memset(ones_row[:], 1.0)
    ones_col = con.tile([128, 1], BF16)
    nc.gpsimd.memset(ones_col[:], 1.0)
    ones_col_f = con.tile([128, 1], F32)
    nc.gpsimd.memset(ones_col_f[:], 1.0)

    iota_w = con.tile([128, W], I32)
    nc.gpsimd.iota(iota_w[:], pattern=[[1, W]], base=0, channel_multiplier=0)
    iota385_i = loss_p.tile([128, W + 1], I32, name="iota385_i", tag="e1")
    nc.gpsimd.iota(iota385_i[:], pattern=[[1, W + 1]], base=0, channel_multiplier=0)
    iota385_f = con.tile([128, W + 1], F32)
    nc.vector.tensor_copy(iota385_f[:], iota385_i[:])
    iotam1_i = loss_p.tile([1, W + 1], I32, name="iotam1_i", tag="e1")
    nc.gpsimd.iota(iotam1_i[:], pattern=[[1, W + 1]], base=-1, channel_multiplier=0)
    iotam1_f = con.tile([1, W + 1], F32)
    nc.vector.tensor_copy(iotam1_f[:], iotam1_i[:])
    iota_p = []
    for c in range(TC):
        ip_i = con.tile([128, 1], I32, name=f"ip_i{c}", tag=f"ip_i{c}")
        nc.gpsimd.iota(ip_i[:], pattern=[[0, 1]], base=128 * c, channel_multiplier=1)
        ip_f = con.tile([128, 1], F32, name=f"ip_f{c}", tag=f"ip_f{c}")
        nc.vector.tensor_copy(ip_f[:], ip_i[:])
        iota_p.append(ip_f)

    NM12 = con.tile([128, NB * TC], F32)
    M12 = con.tile([128, NB * TC], F32)

    # ---------------- weights / biases to SBUF
    wqk_t = []
    for k in range(KD):
        w_ = wbig.tile([128, 2 * H * DHP], BF16, name=f"wqk{k}", tag="wbig")
        nc.sync.dma_start(w_[:], t['wqk'][128 * k:128 * (k + 1), :])
        wqk_t.append(w_)
    wv_t = []
    for k in range(KD):
        w_ = wvp.tile([128, H * DHP], BF16, name=f"wv{k}", tag="wv")
        nc.sync.dma_start(w_[:], t['wv'][128 * k:128 * (k + 1), :])
        wv_t.append(w_)

    bias = {}
    for name, n, dt in (('bqk', 16, F32), ('b1', 16, F32), ('bo', 6, F32),
                        ('b2', 6, F32), ('root', 6, BF16), ('uw', 6, BF16)):
        b_ = con.tile([128, n], dt, name=f"bc_{name}", tag=f"bc_{name}")
        nc.sync.dma_start(b_[:], t[name].rearrange("(n p) -> p n", p=128))
        bias[name] = b_
    bv_row = con.tile([1, H * DHP], BF16)
    nc.sync.dma_start(bv_row[:], t['bv'][None, :])
    ub_t = con.tile([1, 1], F32)
    nc.sync.dma_start(ub_t[:], t['ub'][:, :])

    # persistent per-batch tiles
    X = [[None] * KD for _ in range(NB)]
    cneg_b = [None] * NB
    gold_f = [None] * NB

    # ================ P0: pool (segment mean) per batch ================
    for b in range(NB):
        wid_i = tmp_p.tile([128, SC], I32, name=f"wid_i{b}", tag="wid_i")
        nc.sync.dma_start(wid_i[:], t['wid'][b].rearrange("(c p) -> p c", p=128))
        mx_i = tmp_p.tile([1, 1], I32, name=f"mx_i{b}", tag="mx_i")
        nc.sync.dma_start(mx_i[:], t['wid'][b:b + 1, S - 1:S])
        mx_f = tmp_p.tile([1, 1], F32, name=f"mx_f{b}", tag="mx_f")
        nc.vector.tensor_copy(mx_f[:], mx_i[:])

        g_i = tmp_p.tile([128, TC], I32, name=f"g_i{b}", tag="g_i")
        nc.sync.dma_start(g_i[:], t['gold'][b].rearrange("(c p) -> p c", p=128))
        gf = batch_rows.tile([128, TC], F32, name=f"gold_f{b}", tag="gold_f")
        nc.vector.tensor_copy(gf[:], g_i[:])
        gold_f[b] = gf

        lh_t, oh_t = [], []
        for s in range(SC):
            lh_ = lhp.tile([128, D], BF16, name=f"lh{b}_{s}", tag="lh")
            nc.sync.dma_start(lh_[:], t['lh'][b, 128 * s:128 * (s + 1), :])
            lh_t.append(lh_)
            oh_ = ohp.tile([128, W], BF16, name=f"oh{b}_{s}", tag="oh")
            nc.vector.tensor_tensor(
                out=oh_[:], in0=wid_i[:, s:s + 1].to_broadcast([128, W]),
                in1=iota_w[:], op=ALU.is_equal)
            oh_t.append(oh_)

        cnts = ps_mm.tile([1, W], F32, name=f"cnts{b}", tag="ps_mm")
        for s in range(SC):
            nc.tensor.matmul(cnts[:], lhsT=ones_col[:], rhs=oh_t[s][:],
                             start=(s == 0), stop=(s == SC - 1))
        c1 = rows.tile([1, W], F32, name=f"c1_{b}", tag="rowf")
        nc.vector.tensor_scalar_max(c1[:], cnts[:], 1.0)
        rcp = rows.tile([1, W], F32, name=f"rcp{b}", tag="rowf")
        nc.vector.reciprocal(rcp[:], c1[:])
        rb = bcp.tile([128, W], F32, name=f"rb{b}", tag="bc")
        nc.gpsimd.partition_broadcast(rb[:], rcp[:])

        for d in range(KD):
            sums = ps_acc.tile([128, W], F32, name=f"sums{b}_{d}", tag="ps_acc")
            for s in range(SC):
                nc.tensor.matmul(sums[:], lhsT=lh_t[s][:, 128 * d:128 * (d + 1)],
                                 rhs=oh_t[s][:], start=(s == 0), stop=(s == SC - 1))
            x_ = xfam.tile([128, W], BF16, name=f"X{b}_{d}", tag="xfam")
            nc.vector.tensor_tensor(out=x_[:], in0=sums[:], in1=rb[:], op=ALU.mult)
            X[b][d] = x_

        # maxid, masks, cneg
        maxid = tmp_p.tile([128, 1], F32, name=f"maxid{b}", tag="maxid")
        nc.gpsimd.partition_broadcast(maxid[:], mx_f[:])
        for c in range(TC):
            nc.vector.tensor_tensor(out=M12[:, TC * b + c:TC * b + c + 1],
                                    in0=iota_p[c][:], in1=maxid[:], op=ALU.is_le)
        ct = rows.tile([1, W + 1], F32, name=f"ct{b}", tag="rowf")
        nc.vector.tensor_tensor(out=ct[:], in0=iotam1_f[:],
                                in1=maxid[0:1, 0:1].to_broadcast([1, W + 1]),
                                op=ALU.is_gt)
        cr = rows.tile([1, W + 1], F32, name=f"cr{b}", tag="rowf")
        nc.vector.tensor_scalar_mul(cr[:], ct[:], NEG)
        cb = batch_rows.tile([128, W + 1], F32, name=f"cneg{b}", tag="cneg")
        nc.gpsimd.partition_broadcast(cb[:], cr[:])
        cneg_b[b] = cb

    if STAGE <= 1:
        _probe(X[NB - 1][KD - 1][0:1, 0:2])
        return

    # ================ helper: LN (feature-major) -> z tiles ================
    def layer_norm(xt, b, label):
        s1 = ps_mm.tile([1, W], F32, name=f"s1{label}{b}", tag="ps_mm")
        for k in range(KD):
            nc.tensor.matmul(s1[:], lhsT=ones_col[:], rhs=xt[k][:],
                             start=(k == 0), stop=(k == KD - 1))
        s2 = ps_mm.tile([1, W], F32, name=f"s2{label}{b}", tag="ps_mm")
        for k in range(KD):
            sq = sqp.tile([128, W], BF16, name=f"sq{label}{b}_{k}", tag="sq")
            nc.scalar.activation(sq[:], xt[k][:], AF.Square)
            nc.tensor.matmul(s2[:], lhsT=ones_col[:], rhs=sq[:],
                             start=(k == 0), stop=(k == KD - 1))
        mean = rows.tile([1, W], F32, name=f"mean{label}{b}", tag="lnrow")
        nc.vector.tensor_scalar_mul(mean[:], s1[:], 1.0 / D)
        v = rows.tile([1, W], F32, name=f"v{label}{b}", tag="lnrow")
        nc.vector.tensor_scalar_mul(v[:], s2[:], 1.0 / D)
        m2 = rows.tile([1, W], F32, name=f"m2{label}{b}", tag="lnrow")
        nc.vector.tensor_tensor(out=m2[:], in0=mean[:], in1=mean[:], op=ALU.mult)
        nc.vector.tensor_tensor(out=v[:], in0=v[:], in1=m2[:], op=ALU.subtract)
        nc.vector.tensor_scalar_add(v[:], v[:], 1e-5)
        r = rows.tile([1, W], F32, name=f"r{label}{b}", tag="lnrow")
        nc.vector.reciprocal(r[:], v[:])
        rstd = rows.tile([1, W], F32, name=f"rstd{label}{b}", tag="lnrow")
        nc.scalar.activation(rstd[:], r[:], AF.Sqrt)
        nc.vector.tensor_tensor(out=mean[:], in0=mean[:], in1=rstd[:], op=ALU.mult)
        rstd_b = bcp.tile([128, W], F32, name=f"rstdB{label}{b}", tag="bc")
        nc.gpsimd.partition_broadcast(rstd_b[:], rstd[:])
        mpr_b = bcp.tile([128, W], F32, name=f"mprB{label}{b}", tag="bc")
        nc.gpsimd.partition_broadcast(mpr_b[:], mean[:])
        z = []
        for k in range(KD):
            zt = zp.tile([128, W], BF16, name=f"z{label}{b}_{k}", tag="z")
            tt = tmp_p.tile([128, W], BF16, name=f"zt{label}{b}_{k}", tag="ztmp")
            nc.vector.tensor_tensor(out=tt[:], in0=xt[k][:], in1=rstd_b[:], op=ALU.mult)
            nc.vector.tensor_tensor(out=zt[:], in0=tt[:], in1=mpr_b[:], op=ALU.subtract)
            z.append(zt)
        return z

    # ================ P1-P4: LN1, QKV, attention, Wo per batch ================
    wo_t = []
    for k in range(H):
        w_ = wst.tile([128, D], BF16, name=f"wo{k}", tag="wst")
        nc.sync.dma_start(w_[:], t['wo'][128 * k:128 * (k + 1), :])
        wo_t.append(w_)
    X2 = [[None] * KD for _ in range(NB)]
    for b in range(NB):
        z = layer_norm(X[b], b, "A")

        # V' token-major [128t, 1024] x3
        vt = []
        for c in range(TC):
            v_ = vtp.tile([128, H * DHP], BF16, name=f"V{b}_{c}", tag="vt")
            for n in range(2):
                cs = slice(512 * n, 512 * (n + 1))
                vp = ps_mm.tile([128, 512], F32, name=f"vp{b}_{c}_{n}", tag="ps_mm")
                for k in range(KD):
                    nc.tensor.matmul(vp[:], lhsT=z[k][:, 128 * c:128 * (c + 1)],
                                     rhs=wv_t[k][:, cs], start=(k == 0), stop=False)
                nc.tensor.matmul(vp[:], lhsT=ones_row[:], rhs=bv_row[:, cs],
                                 start=False, stop=True)
                nc.scalar.copy(v_[:, cs], vp[:])
            vt.append(v_)

        # heads: q' (m=h), k' (m=8+h), then attention for head h
        y = []
        for h in range(H):
            qk = []
            for m in (h, H + h):
                qp = ps_mm.tile([128, W], F32, name=f"qp{b}_{m}", tag="ps_mm")
                for k in range(KD):
                    nc.tensor.matmul(qp[:], lhsT=wqk_t[k][:, 128 * m:128 * (m + 1)],
                                     rhs=z[k][:], start=(k == 0), stop=(k == KD - 1))
                qs = qkp.tile([128, W], BF16, name=f"qk{b}_{m}", tag="qk")
                nc.scalar.activation(qs[:], qp[:], AF.Identity,
                                     bias=bias['bqk'][:, m:m + 1])
                qk.append(qs)
            q_t, k_t = qk

            ex = []
            for c in range(TC):
                sp = ps_mm.tile([128, W], F32, name=f"sp{b}_{h}_{c}", tag="ps_mm")
                nc.tensor.matmul(sp[:], lhsT=k_t[:, 128 * c:128 * (c + 1)],
                                 rhs=q_t[:], start=True, stop=True)
                e_ = exp_p.tile([128, W], BF16, name=f"ex{b}_{h}_{c}", tag="ex")
                nc.scalar.activation(e_[:], sp[:], AF.Exp)
                ex.append(e_)

            yraw = ps_acc.tile([128, W], F32, name=f"yraw{b}_{h}", tag="ps_acc")
            for c in range(TC):
                nc.tensor.matmul(yraw[:], lhsT=vt[c][:, DHP * h:DHP * (h + 1)],
                                 rhs=ex[c][:], start=(c == 0), stop=(c == TC - 1))
            rcp = rows.tile([1, W], F32, name=f"arcp{b}_{h}", tag="rowf")
            nc.vector.reciprocal(rcp[:], yraw[DH:DH + 1, :])
            rb = bcp.tile([128, W], F32, name=f"arb{b}_{h}", tag="bc")
            nc.gpsimd.partition_broadcast(rb[:], rcp[:])
            y_ = yp.tile([128, W], BF16, name=f"y{b}_{h}", tag="y")
            nc.vector.tensor_tensor(out=y_[:], in0=yraw[:], in1=rb[:], op=ALU.mult)
            y.append(y_)

        # Wo' + residual
        for m in range(KD):
            op = ps_mm.tile([128, W], F32, name=f"op{b}_{m}", tag="ps_mm")
            for k in range(H):
                nc.tensor.matmul(op[:], lhsT=wo_t[k][:, 128 * m:128 * (m + 1)],
                                 rhs=y[k][:], start=(k == 0), stop=(k == H - 1))
            tt = tmp_p.tile([128, W], BF16, name=f"ot{b}_{m}", tag="ztmp")
            nc.scalar.activation(tt[:], op[:], AF.Identity, bias=bias['bo'][:, m:m + 1])
            x2 = xfam.tile([128, W], BF16, name=f"X2_{b}_{m}", tag="xfam")
            nc.vector.tensor_tensor(out=x2[:], in0=tt[:], in1=X[b][m][:], op=ALU.add)
            X2[b][m] = x2

    if STAGE <= 2:
        _probe(X2[NB - 1][KD - 1][0:1, 0:2])
        return

    # ================ P5: LN2 + FFN per batch ================
    w1_t = []
    for k in range(KD):
        w_ = wbig.tile([128, FF], BF16, name=f"w1_{k}", tag="wbig")
        nc.sync.dma_start(w_[:], t['w1'][128 * k:128 * (k + 1), :])
        w1_t.append(w_)
    w2_t = []
    for m in range(FF // 128):
        w_ = wst.tile([128, D], BF16, name=f"w2_{m}", tag="wst")
        nc.sync.dma_start(w_[:], t['w2'][128 * m:128 * (m + 1), :])
        w2_t.append(w_)

    X3 = [[None] * KD for _ in range(NB)]
    for b in range(NB):
        z2 = layer_norm(X2[b], b, "B")
        x3p = []
        for m2 in range(KD):
            x3p.append(ps_acc.tile([128, W], F32, name=f"x3p{b}_{m2}", tag="ps_acc"))
        for m in range(FF // 128):
            w2_ = w2_t[m]
            wp = ps_mm.tile([128, W], F32, name=f"wp{b}_{m}", tag="ps_mm")
            for k in range(KD):
                nc.tensor.matmul(wp[:], lhsT=w1_t[k][:, 128 * m:128 * (m + 1)],
                                 rhs=z2[k][:], start=(k == 0), stop=(k == KD - 1))
            g_ = gp.tile([128, W], BF16, name=f"G{b}_{m}", tag="g")
            nc.scalar.activation(g_[:], wp[:], AF.Gelu, bias=bias['b1'][:, m:m + 1])
            for m2 in range(KD):
                nc.tensor.matmul(x3p[m2][:], lhsT=w2_[:, 128 * m2:128 * (m2 + 1)],
                                 rhs=g_[:], start=(m == 0), stop=(m == FF // 128 - 1))
        for m2 in range(KD):
            tt = tmp_p.tile([128, W], BF16, name=f"ft{b}_{m2}", tag="ztmp")
            nc.scalar.activation(tt[:], x3p[m2][:], AF.Identity,
                                 bias=bias['b2'][:, m2:m2 + 1])
            x3 = xfam.tile([128, W], BF16, name=f"X3_{b}_{m2}", tag="xfam")
            nc.vector.tensor_tensor(out=x3[:], in0=tt[:], in1=X2[b][m2][:], op=ALU.add)
            X3[b][m2] = x3

    if STAGE <= 3:
        _probe(X3[NB - 1][KD - 1][0:1, 0:2])
        return

    # ================ P6-P7: biaffine + loss per batch ================
    wbi_t = []
    for k in range(KD):
        w_ = wst.tile([128, D], BF16, name=f"wbi{k}", tag="wst")
        nc.sync.dma_start(w_[:], t['wbi'][128 * k:128 * (k + 1), :])
        wbi_t.append(w_)

    for b in range(NB):
        t1 = []
        for m in range(KD):
            bp = ps_mm.tile([128, W], F32, name=f"bp{b}_{m}", tag="ps_mm")
            for k in range(KD):
                nc.tensor.matmul(bp[:], lhsT=wbi_t[k][:, 128 * m:128 * (m + 1)],
                                 rhs=X3[b][k][:], start=(k == 0), stop=(k == KD - 1))
            t1_ = t1p.tile([128, W], BF16, name=f"T1_{b}_{m}", tag="t1")
            nc.scalar.copy(t1_[:], bp[:])
            t1.append(t1_)

        up0 = ps_mm.tile([1, 1], F32, name=f"up0{b}", tag="ps_mm")
        for k in range(KD):
            nc.tensor.matmul(up0[:], lhsT=bias['uw'][:, k:k + 1],
                             rhs=bias['root'][:, k:k + 1],
                             start=(k == 0), stop=(k == KD - 1))
        upx = ps_mm.tile([1, W], F32, name=f"upx{b}", tag="ps_mm")
        for k in range(KD):
            nc.tensor.matmul(upx[:], lhsT=bias['uw'][:, k:k + 1],
                             rhs=X3[b][k][:], start=(k == 0), stop=(k == KD - 1))
        u_f = rows.tile([1, W + 1], F32, name=f"uf{b}", tag="rowf")
        nc.vector.tensor_scalar_add(u_f[:, 0:1], up0[:], ub_t[0:1, 0:1])
        nc.vector.tensor_scalar_add(u_f[:, 1:W + 1], upx[:], ub_t[0:1, 0:1])
        u_bf = rows.tile([1, W + 1], BF16, name=f"ub{b}", tag="rowb")
        nc.vector.tensor_copy(u_bf[:], u_f[:])
        if STAGE == 31:
            _probe(u_f[0:1, 0:2])
            return

        for c in range(TC):
            L = ps_mm.tile([128, W + 1], F32, name=f"L{b}_{c}", tag="ps_mm")
            nc.tensor.matmul(L[:, :], lhsT=ones_row[:], rhs=u_bf[:],
                             start=True, stop=False)
            for k in range(KD):
                nc.tensor.matmul(L[:, 0:1], lhsT=t1[k][:, 128 * c:128 * (c + 1)],
                                 rhs=bias['root'][:, k:k + 1],
                                 start=False, stop=False)
            for k in range(KD):
                nc.tensor.matmul(L[:, 1:W + 1],
                                 lhsT=t1[k][:, 128 * c:128 * (c + 1)],
                                 rhs=X3[b][k][:], start=False, stop=(k == KD - 1))

            Lm = loss_p.tile([128, W + 1], F32, name=f"Lm{b}_{c}", tag="lm")
            nc.vector.tensor_tensor(out=Lm[:], in0=L[:], in1=cneg_b[b][:], op=ALU.add)
            if STAGE == 32:
                _probe(Lm[0:1, 0:2])
                return
            mx = rows.tile([128, 1], F32, name=f"mx{b}_{c}", tag="colf", bufs=8)
            nc.vector.tensor_reduce(out=mx[:], in_=Lm[:], axis=AX.X, op=ALU.max)
            nmx = rows.tile([128, 1], F32, name=f"nmx{b}_{c}", tag="colf", bufs=8)
            nc.scalar.mul(nmx[:], mx[:], -1.0)
            E = loss_p.tile([128, W + 1], F32, name=f"E{b}_{c}", tag="e1")
            Ssum = rows.tile([128, 1], F32, name=f"S{b}_{c}", tag="colf", bufs=8)
            nc.scalar.activation(E[:], Lm[:], AF.Exp, bias=nmx[:], accum_out=Ssum[:])
            lnS = rows.tile([128, 1], F32, name=f"lnS{b}_{c}", tag="colf", bufs=8)
            nc.scalar.activation(lnS[:], Ssum[:], AF.Ln)
            if STAGE == 33:
                _probe(lnS[0:1, 0:1])
                return
            oneh = loss_p.tile([128, W + 1], F32, name=f"oneh{b}_{c}", tag="lm")
            nc.vector.tensor_tensor(
                out=oneh[:], in0=iota385_f[:],
                in1=gold_f[b][:, c:c + 1].to_broadcast([128, W + 1]), op=ALU.is_equal)
            E2 = loss_p.tile([128, W + 1], F32, name=f"E2{b}_{c}", tag="e1")
            picked = rows.tile([128, 1], F32, name=f"pk{b}_{c}", tag="colf", bufs=8)
            nc.vector.tensor_tensor(out=E2[:], in0=Lm[:], in1=oneh[:], op=ALU.mult)
            nc.vector.tensor_reduce(out=picked[:], in_=E2[:], axis=AX.X, op=ALU.add)
            if STAGE == 34:
                _probe(picked[0:1, 0:1])
                return
            t1_ = rows.tile([128, 1], F32, name=f"nt{b}_{c}", tag="colf", bufs=8)
            nc.vector.tensor_tensor(out=t1_[:], in0=mx[:], in1=lnS[:], op=ALU.add)
            nll = rows.tile([128, 1], F32, name=f"nll{b}_{c}", tag="colf", bufs=8)
            nc.vector.tensor_tensor(out=nll[:], in0=t1_[:], in1=picked[:],
                                    op=ALU.subtract)
            j = TC * b + c
            nc.vector.tensor_tensor(out=NM12[:, j:j + 1], in0=nll[:],
                                    in1=M12[:, j:j + 1], op=ALU.mult)

    if STAGE <= 4:
        _probe(NM12[0:1, 0:2])
        return

    # ================ P8: final reduction (exact fp32 matmul) ================
    out_sb = con.tile([1, 2], F32)
    fp1 = ps_mm.tile([1, NB * TC], F32, name="fp1", tag="ps_mm")
    nc.tensor.matmul(fp1[:], lhsT=ones_col_f[:], rhs=NM12[:], start=True, stop=True)
    nc.vector.tensor_reduce(out=out_sb[:, 0:1], in_=fp1[:], axis=AX.X, op=ALU.add)
    fp2 = ps_mm.tile([1, NB * TC], F32, name="fp2", tag="ps_mm")
    nc.tensor.matmul(fp2[:], lhsT=ones_col_f[:], rhs=M12[:], start=True, stop=True)
    nc.vector.tensor_reduce(out=out_sb[:, 1:2], in_=fp2[:], axis=AX.X, op=ALU.add)
    nc.sync.dma_start(t['out'][:, :], out_sb[:])


# ---------------------------------------------------------------- driver

_CACHE = {}


def build_nc():
    if 'nc' in _CACHE:
        return _CACHE['nc']
    nc = bacc.Bacc("TRN2", target_bir_lowering=False, debug=False)
    t = _declare(nc)
    with tile.TileContext(nc) as tc_:
        _build_body(nc, tc_, t)
    nc.compile()
    _CACHE['nc'] = nc
    return nc


def kernel(**inputs):
    nc = build_nc()
    host = _prep_host(inputs)
    bf16 = ml_dtypes.bfloat16
    lh = np.asarray(inputs['last_hidden'], np.float32).astype(bf16)
    wid = np.asarray(inputs['word_ids'], np.int32)
    gold = np.asarray(inputs['heads_gold'], np.int32)

    in_maps = []
    for c in range(NCORES):
        sl = slice(c * NB, (c + 1) * NB)
        m = {'lh': lh[sl], 'wid': wid[sl], 'gold': gold[sl]}
        m.update(host)
        in_maps.append(m)

    res = run_bass_kernel_spmd(nc, in_maps, core_ids=list(range(NCORES)))
    num = 0.0
    den = 0.0
    for c in range(NCORES):
        o = res.results[c]['out']
        num += float(o[0, 0])
        den += float(o[0, 1])
    return np.float32(num / den)


if __name__ == '__main__':
    build_nc()
    print("build + compile OK")
